# revision 23
# baseline (speedup 1.0000x reference)
"""Block-sparse matmul + bias + relu on 8 Trainium2 NeuronCores.

Strategy (data-parallel over batch):
  - Shard x along batch: 8 cores x 512 rows. w_blocks/bias replicated.
  - Per core, compute out^T with the PE in 32x32 tiling mode:
      * x^T resident in SBUF as [128, 32, 512]: input block i lives at
        partitions 32*(i%4) .. 32*(i%4)+31, free tile i//4.
      * each nonzero block (i,j) is one matmul: lhsT = w_block [K=32, M=32],
        rhs = x^T block i [32, 512], accumulated into PSUM at partition strip
        32*(j%4) of bank (i%4): tile_position=(32*(i%4), 32*(j%4)).
      * output block-cols processed in quads (4 cols -> 4 strips x 4 banks),
        16 PE tiles run concurrently.
  - Per quad: DVE sums banks 0+1 and (2+bias)+3, GPSIMD sums the pair
    results, ACT applies relu + bf16-converts into an 8-quad staging tile;
    one DMA per 8 quads.
  - DMA: one large DMA per input tensor; x/out on the SP HWDGE ring, w/bias
    on the ACT HWDGE ring. Loop body is 2x unrolled with double-buffered
    inputs so iteration k+1's loads overlap iteration k's compute.
  - Host: transpose/cast prep (bf16 feeds the PE; fp32 accumulate in PSUM).
"""

import contextlib
import os

import numpy as np
import ml_dtypes

import concourse.bass as bass
import concourse.tile as tile
from concourse import mybir
from concourse.bass_utils import run_bass_kernel_spmd

LAST_RESULTS = None  # test-only: BassKernelResults of the last run

BS = 32
KB = 128
NB = 128
BATCH = 4096
NCORES = 8
BC = BATCH // NCORES          # 512 batch rows per core
NQ = NB // 4                  # 32 quads of output block-cols
QG = 8                        # quads per output staging group
IN_DT = mybir.dt.bfloat16
IN_NP = ml_dtypes.bfloat16
OUT_DT = mybir.dt.bfloat16
OUT_NP = ml_dtypes.bfloat16

_CACHE = {}


def _build_schedule_quads(row_idx, col_idx):
    """Schedule: per quad, round-robin emission over the 16 (strip, rowgrp)
    FIFOs. Returns (sched, S, slot_of, dummy_slots); sched[q] is a list of
    (r, c, t, slot, start, stop). Dummy (zero-weight) entries keep every
    (c, r) PSUM region defined."""
    nnz = len(row_idx)
    fifos = [[[[] for _ in range(4)] for _ in range(4)] for _ in range(NQ)]
    for n in range(nnz):
        i = int(row_idx[n]); j = int(col_idx[n])
        fifos[j // 4][j % 4][i % 4].append(n)

    slot_ctr = [0, 0, 0, 0]           # per row-group strip
    sched = []
    slot_of = {}                      # block id -> slot (in its strip)
    dummy_slots = []                  # (r, slot) zero-weight slots
    for q in range(NQ):
        maxlen = 0
        for c in range(4):
            for r in range(4):
                if not fifos[q][c][r]:
                    fifos[q][c][r].append(None)
                maxlen = max(maxlen, len(fifos[q][c][r]))
        emitted = []
        # r cycles fastest: consecutive MMs hit different row groups so the
        # PE can pull the next LDWEIGHTS ahead of in-flight MATMULs.
        for s in range(maxlen):
            for c in range(4):
                for r in range(4):
                    lst = fifos[q][c][r]
                    if s < len(lst):
                        n = lst[s]
                        slot = slot_ctr[r]; slot_ctr[r] += 1
                        if n is None:
                            dummy_slots.append((r, slot))
                            t = 0
                        else:
                            slot_of[n] = slot
                            t = int(row_idx[n]) // 4
                        emitted.append([r, c, t, slot, False, False])
        first_seen = set()
        for e in emitted:
            key = (e[0], e[1])
            if key not in first_seen:
                e[4] = True
                first_seen.add(key)
        last_idx = {}
        for k, e in enumerate(emitted):
            last_idx[(e[0], e[1])] = k
        for k in last_idx.values():
            emitted[k][5] = True
        sched.append([tuple(e) for e in emitted])
    S = max(slot_ctr)
    return sched, S, slot_of, dummy_slots


LAG = 2      # slots between successive row-group phases of the same quad


def _build_schedule_lag4(row_idx, col_idx):
    """Four-phase lagged schedule: row group r of quad q emits in slot
    q + LAG*r, all phases accumulating into ONE PSUM bank per quad. The
    LAG-slot separation guarantees two phases of the same bank never have
    overlapping drains (PE starts MMs strictly in program order), while the
    per-slot round-robin over the four phases keeps all 16 PE tiles busy.
    Returns (slots, S, slot_of, dummy_slots); slots[k] is a list of
    (r, c, t, wslot, start, stop, q)."""
    nnz = len(row_idx)
    fifos = [[[[] for _ in range(4)] for _ in range(4)] for _ in range(NQ)]
    for n in range(nnz):
        i = int(row_idx[n]); j = int(col_idx[n])
        fifos[j // 4][j % 4][i % 4].append(n)

    slot_ctr = [0, 0, 0, 0]
    slot_of = {}
    dummy_slots = []
    groups = {}                  # (q, r) -> ordered entry list
    for q in range(NQ):
        writers = {}             # c -> count (across phases)
        for r in range(4):
            emitted = []
            maxlen = max(len(fifos[q][c][r]) for c in range(4))
            for s in range(maxlen):
                for c in range(4):
                    lst = fifos[q][c][r]
                    if s < len(lst):
                        n = lst[s]
                        wslot = slot_ctr[r]; slot_ctr[r] += 1
                        slot_of[n] = wslot
                        t = int(row_idx[n]) // 4
                        emitted.append([r, c, t, wslot, False, False, q])
                        writers[c] = writers.get(c, 0) + 1
            groups[(q, r)] = emitted
        for c in range(4):
            if c not in writers:
                wslot = slot_ctr[0]; slot_ctr[0] += 1
                dummy_slots.append((0, wslot))
                groups[(q, 0)].append([0, c, 0, wslot, False, False, q])
        # start/stop per strip c in phase-then-emission order
        ordered = [e for r in range(4) for e in groups[(q, r)]]
        seen = set()
        for e in ordered:
            if e[1] not in seen:
                e[4] = True
                seen.add(e[1])
        last = {}
        for idx, e in enumerate(ordered):
            last[e[1]] = idx
        for idx in last.values():
            ordered[idx][5] = True

    nslots = NQ + 3 * LAG
    slots = []
    for k in range(nslots):
        phase_lists = []
        for r in range(4):
            q = k - LAG * r
            if 0 <= q < NQ:
                phase_lists.append(list(groups[(q, r)]))
        cur = []
        idxs = [0] * len(phase_lists)
        while True:
            progressed = False
            for pi, pl in enumerate(phase_lists):
                if idxs[pi] < len(pl):
                    cur.append(tuple(pl[idxs[pi]]))
                    idxs[pi] += 1
                    progressed = True
            if not progressed:
                break
        slots.append(cur)
    S = max(slot_ctr)
    return slots, S, slot_of, dummy_slots


_MULTIWAIT_OK = {"InstDMACopy", "InstUnconditionalBranch",
                 "InstConditionalBranch"}


def _legalize_waits(nc):
    """Engine ISA structs carry a single sync-wait slot; Tile can emit more.
    Offload excess waits onto same-engine NoOps inserted just before the
    instruction (per-engine stream order is the block list order)."""
    ctr = 0
    for f in nc.m.functions:
        for blk in f.blocks:
            out = []
            for inst in blk.instructions:
                si = inst.sync_info
                if (si is not None and si.on_wait and len(si.on_wait) > 1
                        and type(inst).__name__ == "InstDMACopy"):
                    # HWDGE lane sems are monotonic add-only counters; a
                    # DMA's wait on its own completion lane orders it against
                    # unrelated prior DMAs on that lane and is droppable.
                    own = {u.ant_name for u in (si.on_update or [])}
                    keep = [w for w in si.on_wait if w.ant_name not in own]
                    if len(keep) > 1:
                        # Offload all but one wait onto NoOps on the issuing
                        # engine's sequencer, just before the DMA.
                        for w in keep[:-1]:
                            nop = mybir.InstNoOp(name=f"waitnop-{ctr}")
                            ctr += 1
                            nop.engine = inst.engine
                            nop.sync_info = mybir.SyncInfo(on_wait=[w],
                                                           on_update=[])
                            out.append(nop)
                        keep = keep[-1:]
                    inst.sync_info = mybir.SyncInfo(on_wait=keep,
                                                    on_update=si.on_update)
                    out.append(inst)
                    continue
                if (si is not None and si.on_wait and len(si.on_wait) > 1
                        and type(inst).__name__ not in _MULTIWAIT_OK):
                    waits = list(si.on_wait)
                    for w in waits[:-1]:
                        nop = mybir.InstNoOp(name=f"waitnop-{ctr}")
                        ctr += 1
                        nop.engine = inst.engine
                        nop.sync_info = mybir.SyncInfo(on_wait=[w], on_update=[])
                        out.append(nop)
                    inst.sync_info = mybir.SyncInfo(on_wait=[waits[-1]],
                                                    on_update=si.on_update)
                out.append(inst)
            blk.instructions[:] = out


POST = os.environ.get("BASS_KERNEL_POST", "lag4")


def _build_schedule(row_idx, col_idx):
    if POST == "lag4":
        return _build_schedule_lag4(row_idx, col_idx)
    return _build_schedule_quads(row_idx, col_idx)


def _build_program(sched, S, repeat=1, loop_n=0, unroll=2, post=None,
                   ablate=None):
    """One sweep = input DMAs + 32 quads of MMs + fused post + staged output.
    loop_n: wrap `unroll` sweeps in a For_i(loop_n // unroll) hardware loop.
    repeat: straight-line sweeps (used for the real run; unroll ignored).
    All input DMAs for the whole body issue first on the SP ring so output
    DMAs (behind them in FIFO) never delay the next body's prefetch."""
    nc = bass.Bass("TRN2", target_bir_lowering=False, debug=False,
                   num_devices=NCORES)
    x_d = nc.dram_tensor("xt", [128, 32 * BC], IN_DT, kind="ExternalInput").ap()
    w_d = nc.dram_tensor("wim", [128, S * 32], IN_DT, kind="ExternalInput").ap()
    b_d = nc.dram_tensor("bias", [128, 32], mybir.dt.float32,
                         kind="ExternalInput").ap()
    o_d = nc.dram_tensor("outT", [128, NQ * BC], OUT_DT,
                         kind="ExternalOutput").ap()

    if post is None:
        post = POST
    if loop_n:
        assert loop_n % unroll == 0
        n_body = unroll
    else:
        n_body = repeat

    relu = mybir.ActivationFunctionType.Relu
    add = mybir.AluOpType.add
    mx = mybir.AluOpType.max

    with tile.TileContext(nc) as tc:
        loop_cm = (tc.For_i(0, loop_n // unroll, 1) if loop_n
                   else contextlib.nullcontext())
        psum_bufs = 8 if post == "lag4" else 2
        with tc.tile_pool(name="inp", bufs=2) as ipool, \
             tc.tile_pool(name="work", bufs=3) as wpool, \
             tc.tile_pool(name="outs", bufs=2) as opool, \
             tc.tile_pool(name="psum", bufs=psum_bufs, space="PSUM") as ppool, \
             loop_cm:
            ins = []
            for u in range(n_body):
                xt = ipool.tile([128, 32 * BC], IN_DT, tag="xt")
                wt = ipool.tile([128, S * 32], IN_DT, tag="wt")
                bt = ipool.tile([128, 32], mybir.dt.float32, tag="bt")
                nc.sync.dma_start(xt[:], x_d[:])
                nc.sync.dma_start(wt[:], w_d[:])
                nc.sync.dma_start(bt[:], b_d[:])
                ins.append((xt, wt, bt))
            if post == "lag4":
                # one PSUM bank per quad; 4-phase lag-2 MM schedule; single
                # fused DVE (bias+relu) evacuation per quad; staged bf16 out
                for u in range(n_body):
                    xt, wt, bt = ins[u]
                    accs, sts2 = {}, {}
                    nslots = len(sched)
                    for k in range(nslots):
                        for (r, c, t, wslot, start, stop, q) in sched[k]:
                            if q not in accs:
                                acc1 = ppool.tile([128, BC], mybir.dt.float32,
                                                  tag="acc1",
                                                  name=f"acc1_q{q}_u{u}")
                                accs[q] = acc1
                            nc.tensor.matmul(
                                out=accs[q][32 * c:32 * c + 32, :],
                                lhsT=wt[32 * r:32 * r + 32,
                                        wslot * 32:(wslot + 1) * 32],
                                rhs=xt[32 * r:32 * r + 32,
                                       t * BC:(t + 1) * BC],
                                start=start, stop=stop,
                                tile_position=(32 * r, 32 * c),
                                skip_group_check=True,
                            )
                        qd = k - 3 * LAG   # quad whose last phase just ran
                        if 0 <= qd < NQ:
                            if ablate == "mm":
                                e0 = wpool.tile([128, 32], mybir.dt.float32,
                                                tag="e0")
                                nc.vector.tensor_copy(e0[:],
                                                      accs[qd][:, 0:32])
                                continue
                            g, qq = divmod(qd, QG)
                            if qq == 0:
                                st_g = opool.tile([128, QG * BC], OUT_DT,
                                                  tag="st",
                                                  name=f"st_g{g}_u{u}")
                                sts2[g] = st_g
                            st = sts2[g]
                            nc.vector.tensor_scalar(
                                st[:, qq * BC:(qq + 1) * BC], accs[qd][:],
                                bt[:, qd:qd + 1], 0.0, add, mx)
                            if qq == QG - 1:
                                # ACT ring: ACT runs no compute in lag4, so
                                # its sequencer stalling on the DMA's waits
                                # is free, and the SP ring stays clear for
                                # next-iteration input prefetch
                                nc.scalar.dma_start(
                                    o_d[:, g * QG * BC:(g + 1) * QG * BC],
                                    st[:])
            elif post == "pipe6":
                # software-pipelined chain: ACT copies + DVE combines for
                # quad q, ACT relu for quad q-1 (forward-only engine order
                # in each engine's stream; no cross-engine cycles stall it)
                pending = None        # (t3, st_tile, qq, g)
                sts = {}
                for u in range(n_body):
                    xt, wt, bt = ins[u]
                    for q in range(NQ):
                        g, qq = divmod(q, QG)
                        if qq == 0:
                            st_g = opool.tile([128, QG * BC], OUT_DT,
                                              tag="st", name=f"st_g{g}_u{u}")
                            sts[(u, g)] = st_g
                        st = sts[(u, g)]
                        accA = ppool.tile([128, 2 * BC], mybir.dt.float32,
                                          tag="accA", name=f"accA_q{q}_u{u}")
                        accB = ppool.tile([128, 2 * BC], mybir.dt.float32,
                                          tag="accB", name=f"accB_q{q}_u{u}")
                        tiles = {0: (accA, 0), 1: (accA, 1),
                                 2: (accB, 0), 3: (accB, 1)}
                        for (r, c, t, slot, start, stop) in sched[q]:
                            acc, bk = tiles[r]
                            nc.tensor.matmul(
                                out=acc[32 * c:32 * c + 32,
                                        bk * BC:(bk + 1) * BC],
                                lhsT=wt[32 * r:32 * r + 32,
                                        slot * 32:(slot + 1) * 32],
                                rhs=xt[32 * r:32 * r + 32,
                                       t * BC:(t + 1) * BC],
                                start=start, stop=stop,
                                tile_position=(32 * r, 32 * c),
                                skip_group_check=True,
                            )
                        e0 = wpool.tile([128, BC], mybir.dt.float32, tag="e0")
                        e2 = wpool.tile([128, BC], mybir.dt.float32, tag="e2")
                        s1 = wpool.tile([128, BC], mybir.dt.float32, tag="s1")
                        s2 = wpool.tile([128, BC], mybir.dt.float32, tag="s2")
                        t3 = wpool.tile([128, BC], mybir.dt.float32, tag="t3")
                        nc.scalar.copy(e0[:], accA[:, 0:BC])
                        nc.scalar.copy(e2[:], accB[:, 0:BC])
                        nc.vector.scalar_tensor_tensor(
                            s1[:], accA[:, BC:2 * BC], bt[:, q:q + 1], e0[:],
                            add, add)
                        nc.vector.tensor_add(s2[:], accB[:, BC:2 * BC], e2[:])
                        nc.vector.tensor_add(t3[:], s1[:], s2[:])
                        if pending is not None:
                            pt3, pst, pqq, pg, pu = pending
                            nc.scalar.activation(
                                pst[:, pqq * BC:(pqq + 1) * BC], pt3[:], relu)
                            if pqq == QG - 1:
                                nc.sync.dma_start(
                                    o_d[:, pg * QG * BC:(pg + 1) * QG * BC],
                                    pst[:])
                        pending = (t3, st, qq, g, u)
                pt3, pst, pqq, pg, pu = pending
                nc.scalar.activation(
                    pst[:, pqq * BC:(pqq + 1) * BC], pt3[:], relu)
                nc.sync.dma_start(
                    o_d[:, pg * QG * BC:(pg + 1) * QG * BC], pst[:])
            else:
                for u in range(n_body):
                    xt, wt, bt = ins[u]
                    for g in range(NQ // QG):
                        st = opool.tile([128, QG * BC], OUT_DT, tag="st")
                        for qq in range(QG):
                            q = g * QG + qq
                            _emit_quad(nc, sched, q, u, ppool, wpool, xt, wt,
                                       bt, st, qq, post, relu, add)
                        nc.sync.dma_start(
                            o_d[:, g * QG * BC:(g + 1) * QG * BC], st[:])
    _legalize_waits(nc)
    return nc


def _emit_quad(nc, sched, q, u, ppool, wpool, xt, wt, bt, st, qq, post,
               relu, add):
    if post in ("reduce4", "reduce22"):
        accb = ppool.tile([128, 4 * BC], mybir.dt.float32,
                          tag="acc", name=f"acc_q{q}_u{u}")
        tiles = {r: (accb, r) for r in range(4)}
    else:
        accA = ppool.tile([128, 2 * BC], mybir.dt.float32,
                          tag="accA", name=f"accA_q{q}_u{u}")
        accB = ppool.tile([128, 2 * BC], mybir.dt.float32,
                          tag="accB", name=f"accB_q{q}_u{u}")
        tiles = {0: (accA, 0), 1: (accA, 1), 2: (accB, 0), 3: (accB, 1)}
    for (r, c, t, slot, start, stop) in sched[q]:
        acc, bk = tiles[r]
        nc.tensor.matmul(
            out=acc[32 * c:32 * c + 32, bk * BC:(bk + 1) * BC],
            lhsT=wt[32 * r:32 * r + 32, slot * 32:(slot + 1) * 32],
            rhs=xt[32 * r:32 * r + 32, t * BC:(t + 1) * BC],
            start=start, stop=stop,
            tile_position=(32 * r, 32 * c),
            skip_group_check=True,
        )
    out_ap = st[:, qq * BC:(qq + 1) * BC]
    bias_ap = bt[:, q:q + 1]
    if post == "reduce4":
        accb = tiles[0][0]
        rs = wpool.tile([128, BC], mybir.dt.float32, tag="rs")
        nc.vector.tensor_reduce(
            rs[:], accb[:].rearrange("p (b n) -> p n b", b=4),
            mybir.AxisListType.X, add)
        nc.scalar.activation(out_ap, rs[:], relu, bias=bias_ap, scale=1.0)
    elif post == "reduce22":
        accb = tiles[0][0]
        s1 = wpool.tile([128, BC], mybir.dt.float32, tag="s1")
        s2 = wpool.tile([128, BC], mybir.dt.float32, tag="s2")
        t3 = wpool.tile([128, BC], mybir.dt.float32, tag="t3")
        nc.vector.tensor_reduce(
            s1[:], accb[:, 0:2 * BC].rearrange("p (b n) -> p n b", b=2),
            mybir.AxisListType.X, add)
        nc.vector.tensor_reduce(
            s2[:], accb[:, 2 * BC:4 * BC].rearrange("p (b n) -> p n b", b=2),
            mybir.AxisListType.X, add)
        nc.gpsimd.tensor_add(t3[:], s1[:], s2[:])
        nc.scalar.activation(out_ap, t3[:], relu, bias=bias_ap, scale=1.0)
    elif post == "fwd6":
        # forward-only engine order: ACT -> DVE -> GP (no cycles, so each
        # engine's in-order stream pipelines across quads)
        accA, accB = tiles[0][0], tiles[2][0]
        e0 = wpool.tile([128, BC], mybir.dt.float32, tag="e0")
        e2 = wpool.tile([128, BC], mybir.dt.float32, tag="e2")
        s1 = wpool.tile([128, BC], mybir.dt.float32, tag="s1")
        s2 = wpool.tile([128, BC], mybir.dt.float32, tag="s2")
        t3 = wpool.tile([128, BC], mybir.dt.float32, tag="t3")
        nc.scalar.copy(e0[:], accA[:, 0:BC])
        nc.scalar.copy(e2[:], accB[:, 0:BC])
        nc.vector.tensor_add(s1[:], accA[:, BC:2 * BC], e0[:])
        nc.vector.tensor_add(s2[:], accB[:, BC:2 * BC], e2[:])
        nc.gpsimd.tensor_add(t3[:], s1[:], s2[:])
        nc.gpsimd.tensor_scalar(out_ap, t3[:], bias_ap, 0.0,
                                mybir.AluOpType.add, mybir.AluOpType.max)
    elif post == "split6":
        accA, accB = tiles[0][0], tiles[2][0]
        e0 = wpool.tile([128, BC], mybir.dt.float32, tag="e0")
        e2 = wpool.tile([128, BC], mybir.dt.float32, tag="e2")
        s1 = wpool.tile([128, BC], mybir.dt.float32, tag="s1")
        s2 = wpool.tile([128, BC], mybir.dt.float32, tag="s2")
        t3 = wpool.tile([128, BC], OUT_DT, tag="t3")
        nc.scalar.copy(e0[:], accA[:, 0:BC])
        nc.vector.scalar_tensor_tensor(
            s1[:], accA[:, BC:2 * BC], bias_ap, e0[:], add, add)
        nc.scalar.copy(e2[:], accB[:, 0:BC])
        nc.vector.tensor_add(s2[:], accB[:, BC:2 * BC], e2[:])
        nc.gpsimd.tensor_add(t3[:], s1[:], s2[:])
        nc.vector.tensor_scalar_max(out_ap, t3[:], 0.0)
    else:
        raise ValueError(post)


def _prep_inputs(x, w_blocks, bias, row_idx, col_idx, slot_of, dummy_slots, S):
    nnz = len(row_idx)
    # x^T images per core: [128, 32, BC] -> block i at partitions 32*(i%4),
    # free tile i//4.  x[b, 32*(4t+r)+p] -> xt[32r+p, t, b]
    xb = x.astype(IN_NP).reshape(BATCH, 32, 4, 32)        # b, t, r, p
    xt_all = np.ascontiguousarray(xb.transpose(2, 3, 1, 0))  # r, p, t, b
    xt_all = xt_all.reshape(128, 32, BATCH)
    xts = [np.ascontiguousarray(xt_all[:, :, c * BC:(c + 1) * BC]
                                ).reshape(128, 32 * BC) for c in range(NCORES)]
    # w image [128, S*32]
    wim = np.zeros((128, S * 32), dtype=IN_NP)
    wb = w_blocks.astype(IN_NP)
    for n in range(nnz):
        r = int(row_idx[n]) % 4
        s = slot_of[n]
        wim[32 * r:32 * r + 32, 32 * s:32 * s + 32] = wb[n]
    # dummy slots already zero
    bim = np.ascontiguousarray(
        bias.astype(np.float32).reshape(32, 4, 32).transpose(1, 2, 0)
    ).reshape(128, 32)
    return xts, wim, bim


def kernel(x, w_blocks, bias, row_idx, col_idx):
    repeat = int(os.environ.get("BASS_KERNEL_REPEAT", "1"))
    key = (row_idx.tobytes(), col_idx.tobytes(), repeat)
    if key not in _CACHE:
        sched, S, slot_of, dummy_slots = _build_schedule(row_idx, col_idx)
        nc = _build_program(sched, S, repeat=repeat)
        _CACHE[key] = (nc, S, (slot_of, dummy_slots))
    nc, S, aux = _CACHE[key]

    slot_of, dummy_slots = aux
    xts, wim, bim = _prep_inputs(x, w_blocks, bias, row_idx, col_idx,
                                 slot_of, dummy_slots, S)
    in_maps = [{"xt": xts[c], "wim": wim, "bias": bim} for c in range(NCORES)]
    trace = bool(os.environ.get("BASS_KERNEL_TRACE"))
    res = run_bass_kernel_spmd(nc, in_maps, list(range(NCORES)), trace=trace)
    global LAST_RESULTS
    LAST_RESULTS = res

    out = np.empty((BATCH, NB * BS), dtype=np.float32)
    for c in range(NCORES):
        # outT [128, NQ*BC]: partition p = 32*cg + pp (j = 4q + cg),
        # free = q*BC + b; feature f = 32*j + pp = q*128 + cg*32 + pp
        arr = res.results[c]["outT"].astype(np.float32)
        arr = arr.reshape(4, 32, NQ, BC).transpose(3, 2, 0, 1)
        out[c * BC:(c + 1) * BC, :] = arr.reshape(BC, NB * BS)
    return out
